# revision 12
# baseline (speedup 1.0000x reference)
"""Trainium2 Bass kernel for nn_PiGNNLayer (GNN message passing layer).

Contract: kernel(**inputs) takes the FULL inputs (as from setup_inputs())
and returns the FULL [10000, 128] float32 output.

Strategy (8 NeuronCores, SPMD, no collectives):
  - Nodes (and their contiguous K=30-edge blocks) are sharded across the 8
    cores: 1250 nodes / 37500 edges per core.
  - h is replicated to every core's DRAM; the hj gather is an indirect DMA
    (row gather) from DRAM h.
  - All edge activations are kept feature-major [128 feat, edges] so every
    MLP layer is a weight-stationary matmul (lhsT = weight block, rhs
    streams edges). Gathered hj rows arrive edge-major and are transposed
    on the tensor engine (via identity matmul) into feature-major.
  - att_w3 is pre-expanded on the host to A3m = repeat(att_w3, 32, axis=1)
    so the logits matmul directly yields head-broadcast logits on all 128
    partitions; exp(scale*x + b3) then gives the unnormalized attention
    replicated per (head, dim) — no separate broadcast step.
  - Softmax denominators: segmented masked scan (GPSIMD) over exp;
    aggregation: elementwise prod + segmented reduce (DVE); normalization
    is applied after aggregation (sum(att)=1 folds node_b3 into a constant
    to_h_w.T @ node_b3 bias added at the end).
"""

import os
import sys

for _p in ("/opt/trn_rl_repo", "/root/.axon_site/_ro/trn_rl_repo"):
    if os.path.isdir(_p) and _p not in sys.path:
        sys.path.append(_p)

import numpy as np
from ml_dtypes import bfloat16

import concourse.bass as bass
import concourse.bacc as bacc
import concourse.tile as tile
from concourse import mybir
from concourse.bass_utils import run_bass_kernel_spmd

# ---------------------------------------------------------------- constants
N, K, D, H = 10000, 30, 128, 4
DH = D // H
E = N * K
NCORES = 8
NL = N // NCORES            # 1250 nodes per core
EL = NL * K                 # 37500 edges per core
P = 128

UNIT = 1920                 # edges per pipeline unit (64 nodes)
SUB = 960                   # edges per MLP subtile (32 nodes, 2 PSUM banks)
MMCH = 480                  # edge columns per matmul instruction
GTILE = 128                 # rows per gather tile
NGT = (EL + GTILE - 1) // GTILE   # 293 gather tiles per core

F32 = mybir.dt.float32
F32R = mybir.dt.float32r
I32 = mybir.dt.int32

BF16 = mybir.dt.bfloat16
F16 = mybir.dt.float16
_DT = os.environ.get("KERNEL_DTYPE", "f16")
MMDT = {"f16": F16, "bf16": BF16, "f32r": F32R, "f32": F32}[_DT]
SCALE = 1.0 / float(np.sqrt(np.float32(DH)))

# weight block indices inside the packed W tensor [128, 12*128]
WB_A1HI, WB_A1E, WB_A1HJ, WB_A2, WB_A3M, WB_N1E, WB_N1HJ, WB_N2, WB_N3, WB_TH, WB_ID = range(11)
NWB = 11
# bias column indices inside packed B tensor [128, 8]
BC_B1A, BC_B2A, BC_B3BC, BC_B1N, BC_B2N, BC_CTH = range(6)


def _units():
    """per-core unit list: (edge0, n_edges, n_gather_tiles, node0)."""
    units = []
    e0 = 0
    while e0 < EL:
        ue = min(UNIT, EL - e0)
        gt = (ue + GTILE - 1) // GTILE
        units.append((e0, ue, gt, e0 // K))
        e0 += ue
    return units


def build_bass():
    nc = bacc.Bacc("TRN2", target_bir_lowering=False, debug=False,
                   num_devices=NCORES)

    eT_d = nc.dram_tensor("eT", [P, EL], MMDT, kind="ExternalInput")
    h_d = nc.dram_tensor("h", [N, D], MMDT, kind="ExternalInput")
    hTl_d = nc.dram_tensor("hTl", [P, NL], MMDT, kind="ExternalInput")
    idx_d = nc.dram_tensor("idx", [P, NGT], I32, kind="ExternalInput")
    W_d = nc.dram_tensor("W", [P, NWB * 128], MMDT, kind="ExternalInput")
    WF_d = nc.dram_tensor("WF", [P, 256], F32, kind="ExternalInput")
    B_d = nc.dram_tensor("B", [P, 8], F32, kind="ExternalInput")
    smask_d = nc.dram_tensor("smask", [SUB], F32, kind="ExternalInput")
    out_d = nc.dram_tensor("out", [NL, D], F32, kind="ExternalOutput")

    with tile.TileContext(nc) as tc:
        _emit(nc, tc, eT_d, h_d, hTl_d, idx_d, W_d, WF_d, B_d, smask_d, out_d)
    nc.compile()
    return nc


def _emit(nc, tc, eT_d, h_d, hTl_d, idx_d, W_d, WF_d, B_d, smask_d, out_d):
    from contextlib import ExitStack
    ctx = ExitStack()
    with ctx:
        singles = ctx.enter_context(tc.tile_pool(name="singles", bufs=1))
        p_efm = ctx.enter_context(tc.tile_pool(name="p_efm", bufs=2))
        p_hjem = ctx.enter_context(tc.tile_pool(name="p_hjem", bufs=2))
        p_hjfm = ctx.enter_context(tc.tile_pool(name="p_hjfm", bufs=2))
        p_exp = ctx.enter_context(tc.tile_pool(name="p_exp", bufs=2))
        p_sub = ctx.enter_context(tc.tile_pool(name="p_sub", bufs=3))
        p_small = ctx.enter_context(tc.tile_pool(name="p_small", bufs=4))
        p_oem = ctx.enter_context(tc.tile_pool(name="p_oem", bufs=2))
        p_psum = ctx.enter_context(tc.tile_pool(name="p_psum", bufs=1, space="PSUM"))

        # ---- one-time loads
        W_sb = singles.tile([P, NWB * 128], MMDT)
        nc.sync.dma_start(out=W_sb[:, :], in_=W_d[:, :])
        WF_sb = singles.tile([P, 256], F32)
        nc.sync.dma_start(out=WF_sb[:, :], in_=WF_d[:, :])
        B_sb = singles.tile([P, 8], F32)
        nc.sync.dma_start(out=B_sb[:, :], in_=B_d[:, :])
        hTl_sb = singles.tile([P, NL], MMDT)
        nc.sync.dma_start(out=hTl_sb[:, :], in_=hTl_d[:, :])
        idx_sb = singles.tile([P, NGT], I32)
        nc.sync.dma_start(out=idx_sb[:, :], in_=idx_d[:, :])
        smask_sb = singles.tile([P, SUB], F32)
        smask_bc = bass.AP(tensor=smask_d, offset=0, ap=[[0, P], [1, SUB]])
        nc.sync.dma_start(out=smask_sb[:, :], in_=smask_bc)

        out_fm = singles.tile([P, 1280], F32)
        fin_all = singles.tile([P, 1280], F32)

        def Wb(i):
            return W_sb[:, i * 128:(i + 1) * 128]

        def Bc(i):
            return B_sb[:, i:i + 1]

        ident_mm = Wb(WB_ID)          # MMDT identity for hj transposes
        ident_f32 = WF_sb[:, 0:128]   # f32 identity for final transposes
        TH_f32 = WF_sb[:, 128:256]

        # ---- main loop over units
        for (ue0, UE, GT, un0) in _units():
            e_fm = p_efm.tile([P, UNIT], MMDT, tag="efm")
            nc.sync.dma_start(out=e_fm[:, :UE], in_=eT_d[:, ue0:ue0 + UE])

            hj_em = p_hjem.tile([P, UNIT], MMDT, tag="hjem")
            g0 = ue0 // GTILE
            for gt in range(GT):
                nc.gpsimd.indirect_dma_start(
                    out=hj_em[:, gt * 128:(gt + 1) * 128],
                    out_offset=None,
                    in_=h_d[:, :],
                    in_offset=bass.IndirectOffsetOnAxis(
                        ap=idx_sb[:, g0 + gt:g0 + gt + 1], axis=0),
                )

            # transpose gathered rows to feature-major
            hj_fm = p_hjfm.tile([P, UNIT], MMDT, tag="hjfm")
            hj_em_v = hj_em.rearrange("p (t f) -> p t f", f=128)
            for half0 in range(0, GT, 8):
                nt = min(8, GT - half0)
                tp = p_psum.tile([P, 1024], MMDT, tag="tp")
                for t in range(nt):
                    nc.tensor.transpose(
                        out=tp[:, t * 128:(t + 1) * 128],
                        in_=hj_em_v[:, half0 + t, :],
                        identity=ident_mm,
                    )
                nc.vector.tensor_copy(
                    out=hj_fm[:, half0 * 128:half0 * 128 + nt * 128],
                    in_=tp[:, :nt * 128],
                )

            exp_sb = p_exp.tile([P, UNIT], F32, tag="exp")

            # ---- MLP subtiles
            for s0 in range(0, UE, SUB):
                SE = min(SUB, UE - s0)
                SN = SE // K
                sn0 = (ue0 + s0) // K     # global local-node index

                def chunks():
                    c = 0
                    while c < SE:
                        yield c, min(MMCH, SE - c)
                        c += MMCH

                def pcol(c):
                    # edge-chunk start -> psum column (each chunk gets a bank)
                    return (c // MMCH) * 512

                def pv(ps):
                    # psum read view matching a contiguous [P, SE] tensor
                    if SE == SUB:
                        return ps.rearrange("p (b z) -> p b z", z=512)[:, :, :MMCH]
                    return ps[:, :SE]

                def sv(ap):
                    # contiguous SBUF [P, SE] view shaped to match pv()
                    if SE == SUB:
                        return ap.rearrange("p (b z) -> p b z", z=MMCH)
                    return ap

                # --- attention path
                ps1 = p_psum.tile([P, 1024], F32, tag="mm", bufs=3)
                for c, CE in chunks():
                    cn = (s0 + c) // K  # node offset within unit
                    nch = CE // K
                    hi_src = hTl_sb[:, un0 + cn: un0 + cn + nch]
                    hi_bc = bass.AP(tensor=hi_src.tensor, offset=hi_src.offset,
                                    ap=[hi_src.ap[0], hi_src.ap[1], [0, K]])
                    nc.tensor.matmul(ps1[:, pcol(c):pcol(c) + CE], (Wb(WB_A1E)),
                                     (e_fm[:, s0 + c:s0 + c + CE]),
                                     start=True, stop=False)
                    nc.tensor.matmul(ps1[:, pcol(c):pcol(c) + CE], (Wb(WB_A1HJ)),
                                     (hj_fm[:, s0 + c:s0 + c + CE]),
                                     start=False, stop=False)
                    nc.tensor.matmul(ps1[:, pcol(c):pcol(c) + CE], (Wb(WB_A1HI)),
                                     (hi_bc), start=False, stop=True)
                w1 = p_sub.tile([P, SUB], MMDT, tag="w1")
                nc.scalar.activation(sv(w1[:, :SE]), pv(ps1),
                                     mybir.ActivationFunctionType.Relu,
                                     bias=Bc(BC_B1A))
                ps2 = p_psum.tile([P, 1024], F32, tag="mm", bufs=3)
                for c, CE in chunks():
                    nc.tensor.matmul(ps2[:, pcol(c):pcol(c) + CE], (Wb(WB_A2)),
                                     (w1[:, c:c + CE]), start=True, stop=True)
                w2 = p_sub.tile([P, SUB], MMDT, tag="w2")
                nc.scalar.activation(sv(w2[:, :SE]), pv(ps2),
                                     mybir.ActivationFunctionType.Relu,
                                     bias=Bc(BC_B2A))
                ps3 = p_psum.tile([P, 1024], F32, tag="mm", bufs=3)
                for c, CE in chunks():
                    nc.tensor.matmul(ps3[:, pcol(c):pcol(c) + CE], (Wb(WB_A3M)),
                                     (w2[:, c:c + CE]), start=True, stop=True)
                nc.scalar.activation(sv(exp_sb[:, s0:s0 + SE]), pv(ps3),
                                     mybir.ActivationFunctionType.Exp,
                                     bias=Bc(BC_B3BC), scale=SCALE)

                # softmax denominators: segmented reduce over K
                den = p_small.tile([P, 32], F32, tag="den")
                nc.vector.tensor_reduce(
                    out=den[:, :SN],
                    in_=exp_sb[:, s0:s0 + SE].rearrange("p (n k) -> p n k", k=K),
                    axis=mybir.AxisListType.X,
                    op=mybir.AluOpType.add,
                )

                # --- node (value) path
                ps4 = p_psum.tile([P, 1024], F32, tag="mm", bufs=3)
                for c, CE in chunks():
                    nc.tensor.matmul(ps4[:, pcol(c):pcol(c) + CE], (Wb(WB_N1E)),
                                     (e_fm[:, s0 + c:s0 + c + CE]),
                                     start=True, stop=False)
                    nc.tensor.matmul(ps4[:, pcol(c):pcol(c) + CE], (Wb(WB_N1HJ)),
                                     (hj_fm[:, s0 + c:s0 + c + CE]),
                                     start=False, stop=True)
                v1 = p_sub.tile([P, SUB], MMDT, tag="v1")
                nc.scalar.activation(sv(v1[:, :SE]), pv(ps4),
                                     mybir.ActivationFunctionType.Gelu,
                                     bias=Bc(BC_B1N))
                ps5 = p_psum.tile([P, 1024], F32, tag="mm", bufs=3)
                for c, CE in chunks():
                    nc.tensor.matmul(ps5[:, pcol(c):pcol(c) + CE], (Wb(WB_N2)),
                                     (v1[:, c:c + CE]), start=True, stop=True)
                v2 = p_sub.tile([P, SUB], MMDT, tag="v2")
                nc.scalar.activation(sv(v2[:, :SE]), pv(ps5),
                                     mybir.ActivationFunctionType.Gelu,
                                     bias=Bc(BC_B2N))
                ps6 = p_psum.tile([P, 1024], F32, tag="mm", bufs=3)
                for c, CE in chunks():
                    nc.tensor.matmul(ps6[:, pcol(c):pcol(c) + CE], (Wb(WB_N3)),
                                     (v2[:, c:c + CE]), start=True, stop=True)

                # --- weighted aggregation over the K=30 neighbors
                prod = p_sub.tile([P, SUB], F32, tag="prod")
                nc.vector.tensor_tensor(out=sv(prod[:, :SE]),
                                        in0=sv(exp_sb[:, s0:s0 + SE]),
                                        in1=pv(ps6),
                                        op=mybir.AluOpType.mult)
                agg = p_small.tile([P, 32], F32, tag="agg")
                nc.vector.tensor_reduce(
                    out=agg[:, :SN],
                    in_=prod[:, :SE].rearrange("p (n k) -> p n k", k=K),
                    axis=mybir.AxisListType.X,
                    op=mybir.AluOpType.add,
                )
                rec = p_small.tile([P, 32], F32, tag="rec")
                nc.vector.reciprocal(rec[:, :SN], den[:, :SN])
                nc.vector.tensor_tensor(out=out_fm[:, sn0:sn0 + SN],
                                        in0=agg[:, :SN], in1=rec[:, :SN],
                                        op=mybir.AluOpType.mult)

        # ---- final: to_h matmul + bias, transpose back to node-major, store
        for c0 in range(0, NL, 512):
            CN = min(512, NL - c0)
            psf = p_psum.tile([P, 1024], F32, tag="tp", bufs=1)
            nc.tensor.matmul(psf[:, :CN], TH_f32,
                             (out_fm[:, c0:c0 + CN]), start=True, stop=True)
            nc.scalar.activation(fin_all[:, c0:c0 + CN], psf[:, :CN],
                                 mybir.ActivationFunctionType.Identity,
                                 bias=Bc(BC_CTH))

        for t in range((NL + 127) // 128):
            TN = min(128, NL - t * 128)
            pst = p_psum.tile([P, 1024], F32, tag="tp", bufs=1)
            nc.tensor.transpose(out=pst[:, :128],
                                in_=fin_all[:, t * 128:(t + 1) * 128],
                                identity=ident_f32)
            oem = p_oem.tile([P, 128], F32, tag="oem")
            nc.scalar.activation(oem[:TN, :], pst[:TN, :128],
                                 mybir.ActivationFunctionType.Copy)
            nc.sync.dma_start(out=out_d[t * 128:t * 128 + TN, :],
                              in_=oem[:TN, :])


# ------------------------------------------------------------------- host


def host_prep(h, e, edge_index, att_w1, att_b1, att_w2, att_b2, att_w3,
              att_b3, node_w1, node_b1, node_w2, node_b2, node_w3, node_b3,
              to_h_w):
    _mmnp = {"f16": np.float16, "bf16": bfloat16, "f32r": np.float32, "f32": np.float32}[_DT]
    h = np.ascontiguousarray(np.asarray(h, np.float32))
    h_mm = h.astype(_mmnp)
    e = np.asarray(e, np.float32)
    dst = np.asarray(edge_index[1], np.int32)

    W = np.zeros((P, NWB * 128), np.float32)
    att_w1 = np.asarray(att_w1, np.float32)
    node_w1 = np.asarray(node_w1, np.float32)
    W[:, WB_A1HI * 128:(WB_A1HI + 1) * 128] = att_w1[0:128]
    W[:, WB_A1E * 128:(WB_A1E + 1) * 128] = att_w1[128:256]
    W[:, WB_A1HJ * 128:(WB_A1HJ + 1) * 128] = att_w1[256:384]
    W[:, WB_A2 * 128:(WB_A2 + 1) * 128] = np.asarray(att_w2, np.float32)
    W[:, WB_A3M * 128:(WB_A3M + 1) * 128] = np.repeat(
        np.asarray(att_w3, np.float32), DH, axis=1)
    W[:, WB_N1E * 128:(WB_N1E + 1) * 128] = node_w1[0:128]
    W[:, WB_N1HJ * 128:(WB_N1HJ + 1) * 128] = node_w1[128:256]
    W[:, WB_N2 * 128:(WB_N2 + 1) * 128] = np.asarray(node_w2, np.float32)
    W[:, WB_N3 * 128:(WB_N3 + 1) * 128] = np.asarray(node_w3, np.float32)
    W[:, WB_TH * 128:(WB_TH + 1) * 128] = np.asarray(to_h_w, np.float32)
    W[:, WB_ID * 128:(WB_ID + 1) * 128] = np.eye(128, dtype=np.float32)

    B = np.zeros((P, 8), np.float32)
    B[:, BC_B1A] = np.asarray(att_b1, np.float32)
    B[:, BC_B2A] = np.asarray(att_b2, np.float32)
    B[:, BC_B3BC] = np.repeat(np.asarray(att_b3, np.float32), DH)
    B[:, BC_B1N] = np.asarray(node_b1, np.float32)
    B[:, BC_B2N] = np.asarray(node_b2, np.float32)
    B[:, BC_CTH] = np.asarray(to_h_w, np.float32).T @ np.asarray(node_b3, np.float32)

    WF = np.zeros((P, 256), np.float32)
    WF[:, 0:128] = np.eye(128, dtype=np.float32)
    WF[:, 128:256] = np.asarray(to_h_w, np.float32)

    smask = np.ones(SUB, np.float32)
    smask[::K] = 0.0
    W = W.astype(_mmnp)

    in_maps = []
    for c in range(NCORES):
        n0 = c * NL
        e0 = n0 * K
        eT = np.ascontiguousarray(e[e0:e0 + EL].T.astype(_mmnp))
        hTl = np.ascontiguousarray(h_mm[n0:n0 + NL].T)
        flat = np.zeros(GTILE * NGT, np.int32)
        flat[:EL] = dst[e0:e0 + EL]
        idx = np.ascontiguousarray(flat.reshape(NGT, GTILE).T)
        in_maps.append({
            "eT": eT, "h": h_mm, "hTl": hTl, "idx": idx,
            "W": W, "WF": WF, "B": B, "smask": smask,
        })
    return in_maps


_NC_CACHE = None


def _get_nc():
    global _NC_CACHE
    if _NC_CACHE is None:
        _NC_CACHE = build_bass()
    return _NC_CACHE


def kernel(**inputs) -> np.ndarray:
    nc = _get_nc()
    in_maps = host_prep(**inputs)
    res = run_bass_kernel_spmd(nc, in_maps, list(range(NCORES)))
    outs = [np.asarray(res.results[c]["out"], np.float32) for c in range(NCORES)]
    return np.concatenate(outs, axis=0)


if __name__ == "__main__":
    nc = build_bass()
    print("built ok, instructions:",
          sum(len(bb.instructions) for f in nc.m.functions for bb in f.basicblocks)
          if hasattr(nc.m.functions[0], 'basicblocks') else "?")


# revision 16
# speedup vs baseline: 162.6346x; 162.6346x over previous
"""Trainium2 Bass kernel for nn_PiGNNLayer (GNN message passing layer).

Contract: kernel(**inputs) takes the FULL inputs (as from setup_inputs())
and returns the FULL [10000, 128] float32 output.

Strategy (8 NeuronCores, SPMD, no collectives):
  - Nodes (and their contiguous K=30-edge blocks) are sharded across the 8
    cores: 1250 nodes / 37500 edges per core.
  - h is replicated to every core's DRAM; the hj gather is an indirect DMA
    (row gather) from DRAM h.
  - All edge activations are kept feature-major [128 feat, edges] so every
    MLP layer is a weight-stationary matmul (lhsT = weight block, rhs
    streams edges). Gathered hj rows arrive edge-major and are transposed
    on the tensor engine (via identity matmul) into feature-major.
  - att_w3 is pre-expanded on the host to A3m = repeat(att_w3, 32, axis=1)
    so the logits matmul directly yields head-broadcast logits on all 128
    partitions; exp(scale*x + b3) then gives the unnormalized attention
    replicated per (head, dim) — no separate broadcast step.
  - Softmax denominators: segmented masked scan (GPSIMD) over exp;
    aggregation: elementwise prod + segmented reduce (DVE); normalization
    is applied after aggregation (sum(att)=1 folds node_b3 into a constant
    to_h_w.T @ node_b3 bias added at the end).
"""

import os
import sys

for _p in ("/opt/trn_rl_repo", "/root/.axon_site/_ro/trn_rl_repo"):
    if os.path.isdir(_p) and _p not in sys.path:
        sys.path.append(_p)

import numpy as np
from ml_dtypes import bfloat16

import concourse.bass as bass
import concourse.bacc as bacc
import concourse.tile as tile
from concourse import mybir
from concourse.bass_utils import run_bass_kernel_spmd

# ---------------------------------------------------------------- constants
N, K, D, H = 10000, 30, 128, 4
DH = D // H
E = N * K
NCORES = 8
NL = N // NCORES            # 1250 nodes per core
EL = NL * K                 # 37500 edges per core
P = 128

UNIT = 1920                 # edges per pipeline unit (64 nodes)
SUB = 960                   # edges per MLP subtile (32 nodes, 2 PSUM banks)
MMCH = 480                  # edge columns per matmul instruction
GTILE = 128                 # rows per gather tile
ELP = ((EL + 1023) // 1024) * 1024   # 37504: edges padded for dma_gather
NIC = ELP // 16                      # int16 index columns

F32 = mybir.dt.float32
F32R = mybir.dt.float32r
I32 = mybir.dt.int32
I16 = mybir.dt.int16

BF16 = mybir.dt.bfloat16
F16 = mybir.dt.float16
_DT = os.environ.get("KERNEL_DTYPE", "f16")
MMDT = {"f16": F16, "bf16": BF16, "f32r": F32R, "f32": F32}[_DT]
SCALE = 1.0 / float(np.sqrt(np.float32(DH)))

# weight block indices inside the packed W tensor [128, 12*128]
WB_A1HI, WB_A1E, WB_A1HJ, WB_A2, WB_A3M, WB_N1E, WB_N1HJ, WB_N2, WB_N3, WB_TH, WB_ID = range(11)
NWB = 11
# bias column indices inside packed B tensor [128, 8]
BC_B1A, BC_B2A, BC_B3BC, BC_B1N, BC_B2N, BC_CTH = range(6)


def _units():
    """per-core unit list: (edge0, n_edges, n_gather_tiles, node0)."""
    units = []
    e0 = 0
    while e0 < EL:
        ue = min(UNIT, EL - e0)
        gu = ((ue + 1023) // 1024) * 1024 if ue < UNIT else UNIT  # gather count
        units.append((e0, ue, gu, e0 // K))
        e0 += ue
    return units


def build_bass():
    nc = bacc.Bacc("TRN2", target_bir_lowering=False, debug=False,
                   num_devices=NCORES, dynamic_dma_scratch_size=65536)

    eT_d = nc.dram_tensor("eT", [P, EL], MMDT, kind="ExternalInput")
    h_d = nc.dram_tensor("h", [N, D], MMDT, kind="ExternalInput")
    hTl_d = nc.dram_tensor("hTl", [P, NL], MMDT, kind="ExternalInput")
    idx_d = nc.dram_tensor("idx", [P, NIC], I16, kind="ExternalInput")
    W_d = nc.dram_tensor("W", [P, NWB * 128], MMDT, kind="ExternalInput")
    WF_d = nc.dram_tensor("WF", [P, 256], F32, kind="ExternalInput")
    B_d = nc.dram_tensor("B", [P, 8], F32, kind="ExternalInput")
    out_d = nc.dram_tensor("out", [NL, D], F32, kind="ExternalOutput")

    reps = int(os.environ.get("KERNEL_REPS", "1"))
    with tile.TileContext(nc) as tc:
        for _ in range(reps):
            _emit(nc, tc, eT_d, h_d, hTl_d, idx_d, W_d, WF_d, B_d, out_d)
    nc.compile()
    return nc


def _emit(nc, tc, eT_d, h_d, hTl_d, idx_d, W_d, WF_d, B_d, out_d):
    from contextlib import ExitStack
    ctx = ExitStack()
    with ctx:
        singles = ctx.enter_context(tc.tile_pool(name="singles", bufs=1))
        p_efm = ctx.enter_context(tc.tile_pool(name="p_efm", bufs=2))
        p_hjfm = ctx.enter_context(tc.tile_pool(name="p_hjfm", bufs=2))
        p_exp = ctx.enter_context(tc.tile_pool(name="p_exp", bufs=2))
        p_sub = ctx.enter_context(tc.tile_pool(name="p_sub", bufs=3))
        p_small = ctx.enter_context(tc.tile_pool(name="p_small", bufs=4))
        p_oem = ctx.enter_context(tc.tile_pool(name="p_oem", bufs=2))
        p_psum = ctx.enter_context(tc.tile_pool(name="p_psum", bufs=1, space="PSUM"))

        # ---- one-time loads
        W_sb = singles.tile([P, NWB * 128], MMDT)
        nc.sync.dma_start(out=W_sb[:, :], in_=W_d[:, :])
        WF_sb = singles.tile([P, 256], F32)
        nc.sync.dma_start(out=WF_sb[:, :], in_=WF_d[:, :])
        B_sb = singles.tile([P, 8], F32)
        nc.sync.dma_start(out=B_sb[:, :], in_=B_d[:, :])
        hTl_sb = singles.tile([P, NL], MMDT)
        nc.sync.dma_start(out=hTl_sb[:, :], in_=hTl_d[:, :])
        idx_sb = singles.tile([P, NIC], I16)
        nc.sync.dma_start(out=idx_sb[:, :], in_=idx_d[:, :])

        out_fm = singles.tile([P, 1280], F32)
        fin_all = singles.tile([P, 1280], F32)

        def Wb(i):
            return W_sb[:, i * 128:(i + 1) * 128]

        def Bc(i):
            return B_sb[:, i:i + 1]

        ident_mm = Wb(WB_ID)          # MMDT identity for hj transposes
        ident_f32 = WF_sb[:, 0:128]   # f32 identity for final transposes
        TH_f32 = WF_sb[:, 128:256]

        # ---- main loop over units
        for (ue0, UE, GT, un0) in _units():
            e_fm = p_efm.tile([P, UNIT], MMDT, tag="efm")
            nc.sync.dma_start(out=e_fm[:, :UE], in_=eT_d[:, ue0:ue0 + UE])

            # feature-major gather of destination-node rows (DMA transpose),
            # in <=512-index calls (validated SWDGE ring-safe size)
            hj_fm = p_hjfm.tile([P, UNIT], MMDT, tag="hjfm")
            s = 0
            while s < GT:
                gn = min(512, GT - s)
                c16 = (ue0 + s) // 16
                nc.gpsimd.dma_gather(
                    out_ap=hj_fm[:, s:s + gn].rearrange("p (o n) -> p o n", o=1),
                    in_ap=h_d[:, :],
                    idxs_ap=idx_sb[:, c16:c16 + gn // 16],
                    num_idxs=gn,
                    num_idxs_reg=gn,
                    elem_size=128,
                    transpose=True,
                )
                s += gn

            exp_sb = p_exp.tile([P, UNIT], F32, tag="exp")

            # ---- MLP subtiles: att phase first (exp batched), then node
            # phase (gelu batched) — halves ACT table reloads
            subs = []
            for s0 in range(0, UE, SUB):
                SE = min(SUB, UE - s0)
                subs.append((s0, SE))

            def chunks(SE):
                c = 0
                while c < SE:
                    yield c, min(MMCH, SE - c)
                    c += MMCH

            def pcol(c):
                return (c // MMCH) * 512

            def pv(ps, SE):
                if SE == SUB:
                    return ps.rearrange("p (b z) -> p b z", z=512)[:, :, :MMCH]
                return ps[:, :SE]

            def sv(ap, SE):
                if SE == SUB:
                    return ap.rearrange("p (b z) -> p b z", z=MMCH)
                return ap

            recs = []
            for (s0, SE) in subs:
                SN = SE // K
                ps1 = p_psum.tile([P, 1024], F32, tag="mm", bufs=4)
                for c, CE in chunks(SE):
                    cn = (s0 + c) // K
                    nch = CE // K
                    hi_src = hTl_sb[:, un0 + cn: un0 + cn + nch]
                    hi_bc = bass.AP(tensor=hi_src.tensor, offset=hi_src.offset,
                                    ap=[hi_src.ap[0], hi_src.ap[1], [0, K]])
                    nc.tensor.matmul(ps1[:, pcol(c):pcol(c) + CE], (Wb(WB_A1E)),
                                     (e_fm[:, s0 + c:s0 + c + CE]),
                                     start=True, stop=False)
                    nc.tensor.matmul(ps1[:, pcol(c):pcol(c) + CE], (Wb(WB_A1HJ)),
                                     (hj_fm[:, s0 + c:s0 + c + CE]),
                                     start=False, stop=False)
                    nc.tensor.matmul(ps1[:, pcol(c):pcol(c) + CE], (Wb(WB_A1HI)),
                                     (hi_bc), start=False, stop=True)
                w1 = p_sub.tile([P, SUB], MMDT, tag="w1")
                nc.scalar.activation(sv(w1[:, :SE], SE), pv(ps1, SE),
                                     mybir.ActivationFunctionType.Relu,
                                     bias=Bc(BC_B1A))
                ps2 = p_psum.tile([P, 1024], F32, tag="mm", bufs=4)
                for c, CE in chunks(SE):
                    nc.tensor.matmul(ps2[:, pcol(c):pcol(c) + CE], (Wb(WB_A2)),
                                     (w1[:, c:c + CE]), start=True, stop=True)
                w2 = p_sub.tile([P, SUB], MMDT, tag="w2")
                nc.scalar.activation(sv(w2[:, :SE], SE), pv(ps2, SE),
                                     mybir.ActivationFunctionType.Relu,
                                     bias=Bc(BC_B2A))
                ps3 = p_psum.tile([P, 1024], F32, tag="mm", bufs=4)
                for c, CE in chunks(SE):
                    nc.tensor.matmul(ps3[:, pcol(c):pcol(c) + CE], (Wb(WB_A3M)),
                                     (w2[:, c:c + CE]), start=True, stop=True)
                nc.scalar.activation(sv(exp_sb[:, s0:s0 + SE], SE), pv(ps3, SE),
                                     mybir.ActivationFunctionType.Exp,
                                     bias=Bc(BC_B3BC), scale=SCALE)
                den = p_small.tile([P, 32], F32, tag="den")
                nc.vector.tensor_reduce(
                    out=den[:, :SN],
                    in_=exp_sb[:, s0:s0 + SE].rearrange("p (n k) -> p n k", k=K),
                    axis=mybir.AxisListType.X,
                    op=mybir.AluOpType.add,
                )
                rec = p_small.tile([P, 32], F32, tag="rec")
                nc.vector.reciprocal(rec[:, :SN], den[:, :SN])
                recs.append(rec)

            for (s0, SE), rec in zip(subs, recs):
                SN = SE // K
                sn0 = (ue0 + s0) // K
                ps4 = p_psum.tile([P, 1024], F32, tag="mm", bufs=4)
                for c, CE in chunks(SE):
                    nc.tensor.matmul(ps4[:, pcol(c):pcol(c) + CE], (Wb(WB_N1E)),
                                     (e_fm[:, s0 + c:s0 + c + CE]),
                                     start=True, stop=False)
                    nc.tensor.matmul(ps4[:, pcol(c):pcol(c) + CE], (Wb(WB_N1HJ)),
                                     (hj_fm[:, s0 + c:s0 + c + CE]),
                                     start=False, stop=True)
                v1 = p_sub.tile([P, SUB], MMDT, tag="v1")
                nc.scalar.activation(sv(v1[:, :SE], SE), pv(ps4, SE),
                                     mybir.ActivationFunctionType.Gelu,
                                     bias=Bc(BC_B1N))
                ps5 = p_psum.tile([P, 1024], F32, tag="mm", bufs=4)
                for c, CE in chunks(SE):
                    nc.tensor.matmul(ps5[:, pcol(c):pcol(c) + CE], (Wb(WB_N2)),
                                     (v1[:, c:c + CE]), start=True, stop=True)
                v2 = p_sub.tile([P, SUB], MMDT, tag="v2")
                nc.scalar.activation(sv(v2[:, :SE], SE), pv(ps5, SE),
                                     mybir.ActivationFunctionType.Gelu,
                                     bias=Bc(BC_B2N))
                ps6 = p_psum.tile([P, 1024], F32, tag="mm", bufs=4)
                for c, CE in chunks(SE):
                    nc.tensor.matmul(ps6[:, pcol(c):pcol(c) + CE], (Wb(WB_N3)),
                                     (v2[:, c:c + CE]), start=True, stop=True)
                prod = p_sub.tile([P, SUB], F32, tag="prod")
                nc.vector.tensor_tensor(out=sv(prod[:, :SE], SE),
                                        in0=sv(exp_sb[:, s0:s0 + SE], SE),
                                        in1=pv(ps6, SE),
                                        op=mybir.AluOpType.mult)
                agg = p_small.tile([P, 32], F32, tag="agg")
                nc.vector.tensor_reduce(
                    out=agg[:, :SN],
                    in_=prod[:, :SE].rearrange("p (n k) -> p n k", k=K),
                    axis=mybir.AxisListType.X,
                    op=mybir.AluOpType.add,
                )
                nc.vector.tensor_tensor(out=out_fm[:, sn0:sn0 + SN],
                                        in0=agg[:, :SN], in1=rec[:, :SN],
                                        op=mybir.AluOpType.mult)

        # ---- final: to_h matmul + bias, transpose back to node-major, store
        for c0 in range(0, NL, 512):
            CN = min(512, NL - c0)
            psf = p_psum.tile([P, 1024], F32, tag="mm", bufs=4)
            nc.tensor.matmul(psf[:, :CN], TH_f32,
                             (out_fm[:, c0:c0 + CN]), start=True, stop=True)
            nc.scalar.activation(fin_all[:, c0:c0 + CN], psf[:, :CN],
                                 mybir.ActivationFunctionType.Identity,
                                 bias=Bc(BC_CTH))

        for t in range((NL + 127) // 128):
            TN = min(128, NL - t * 128)
            pst = p_psum.tile([P, 1024], F32, tag="mm", bufs=4)
            nc.tensor.transpose(out=pst[:, :128],
                                in_=fin_all[:, t * 128:(t + 1) * 128],
                                identity=ident_f32)
            oem = p_oem.tile([P, 128], F32, tag="oem")
            nc.scalar.activation(oem[:TN, :], pst[:TN, :128],
                                 mybir.ActivationFunctionType.Copy)
            nc.sync.dma_start(out=out_d[t * 128:t * 128 + TN, :],
                              in_=oem[:TN, :])


# ------------------------------------------------------------------- host


def host_prep(h, e, edge_index, att_w1, att_b1, att_w2, att_b2, att_w3,
              att_b3, node_w1, node_b1, node_w2, node_b2, node_w3, node_b3,
              to_h_w):
    _mmnp = {"f16": np.float16, "bf16": bfloat16, "f32r": np.float32, "f32": np.float32}[_DT]
    h = np.ascontiguousarray(np.asarray(h, np.float32))
    h_mm = h.astype(_mmnp)
    e = np.asarray(e, np.float32)
    dst = np.asarray(edge_index[1], np.int32)

    W = np.zeros((P, NWB * 128), np.float32)
    att_w1 = np.asarray(att_w1, np.float32)
    node_w1 = np.asarray(node_w1, np.float32)
    W[:, WB_A1HI * 128:(WB_A1HI + 1) * 128] = att_w1[0:128]
    W[:, WB_A1E * 128:(WB_A1E + 1) * 128] = att_w1[128:256]
    W[:, WB_A1HJ * 128:(WB_A1HJ + 1) * 128] = att_w1[256:384]
    W[:, WB_A2 * 128:(WB_A2 + 1) * 128] = np.asarray(att_w2, np.float32)
    W[:, WB_A3M * 128:(WB_A3M + 1) * 128] = np.repeat(
        np.asarray(att_w3, np.float32), DH, axis=1)
    W[:, WB_N1E * 128:(WB_N1E + 1) * 128] = node_w1[0:128]
    W[:, WB_N1HJ * 128:(WB_N1HJ + 1) * 128] = node_w1[128:256]
    W[:, WB_N2 * 128:(WB_N2 + 1) * 128] = np.asarray(node_w2, np.float32)
    W[:, WB_N3 * 128:(WB_N3 + 1) * 128] = np.asarray(node_w3, np.float32)
    W[:, WB_TH * 128:(WB_TH + 1) * 128] = np.asarray(to_h_w, np.float32)
    W[:, WB_ID * 128:(WB_ID + 1) * 128] = np.eye(128, dtype=np.float32)

    B = np.zeros((P, 8), np.float32)
    B[:, BC_B1A] = np.asarray(att_b1, np.float32)
    B[:, BC_B2A] = np.asarray(att_b2, np.float32)
    B[:, BC_B3BC] = np.repeat(np.asarray(att_b3, np.float32), DH)
    B[:, BC_B1N] = np.asarray(node_b1, np.float32)
    B[:, BC_B2N] = np.asarray(node_b2, np.float32)
    B[:, BC_CTH] = np.asarray(to_h_w, np.float32).T @ np.asarray(node_b3, np.float32)

    WF = np.zeros((P, 256), np.float32)
    WF[:, 0:128] = np.eye(128, dtype=np.float32)
    WF[:, 128:256] = np.asarray(to_h_w, np.float32)

    W = W.astype(_mmnp)

    in_maps = []
    for c in range(NCORES):
        n0 = c * NL
        e0 = n0 * K
        eT = np.ascontiguousarray(e[e0:e0 + EL].T.astype(_mmnp))
        hTl = np.ascontiguousarray(h_mm[n0:n0 + NL].T)
        flat = np.zeros(ELP, np.int16)
        flat[:EL] = dst[e0:e0 + EL].astype(np.int16)
        idx = np.zeros((P, NIC), np.int16)
        for (ue0, UE, GU, _n0) in _units():
            s = 0
            while s < GU:
                gn = min(512, GU - s)
                w = flat[ue0 + s:ue0 + s + gn].reshape(gn // 16, 16).T
                c16 = (ue0 + s) // 16
                for rep in range(8):
                    idx[rep * 16:(rep + 1) * 16, c16:c16 + gn // 16] = w
                s += gn
        idx = np.ascontiguousarray(idx)
        in_maps.append({
            "eT": eT, "h": h_mm, "hTl": hTl, "idx": idx,
            "W": W, "WF": WF, "B": B,
        })
    return in_maps


_NC_CACHE = None


def _get_nc():
    global _NC_CACHE
    if _NC_CACHE is None:
        _NC_CACHE = build_bass()
    return _NC_CACHE


def kernel(**inputs) -> np.ndarray:
    nc = _get_nc()
    in_maps = host_prep(**inputs)
    res = run_bass_kernel_spmd(nc, in_maps, list(range(NCORES)))
    outs = [np.asarray(res.results[c]["out"], np.float32) for c in range(NCORES)]
    return np.concatenate(outs, axis=0)


if __name__ == "__main__":
    nc = build_bass()
    print("built ok, instructions:",
          sum(len(bb.instructions) for f in nc.m.functions for bb in f.basicblocks)
          if hasattr(nc.m.functions[0], 'basicblocks') else "?")
